# revision 24
# baseline (speedup 1.0000x reference)
"""Causal self-attention with bias — Trainium2 Bass kernel, 8-way sharded.

Sharding: core c -> batch b = c//2, heads h in [8*(c%2), 8*(c%2)+8).
Per core: column-split W_attn (QKV for its 8 heads), full attention for
8 (b, h) pairs, row-split W_proj partial product. Host sums the two
partials per batch and adds the (projected) biases.

v3 (on top of the v2 bf16 pipeline):

* QKV projection in fp8-e4m3 DoubleRow with host-side hi/lo error
  compensation: x and 32*W_attn are split on the host into
  hi = fp8(v), lo = fp8(v - hi); the kernel computes
  xh@Wh + xh@Wl + xl@Wh (the lo@lo term is below bf16 noise). Each
  DoubleRow matmul contracts 256 channels at half the per-column cost,
  so the 3-term product runs at 0.75x the bf16 cost with slightly
  better accuracy. The 32x weight scale keeps the lo parts out of the
  e4m3 subnormal range; it is divided back out in the exp scale (for
  q,k) and the V-copy (for v).
* att@V flipped to out[q, d]: lhsT = P^T chunk (128x128, stationary),
  rhs = V_aug (128x65, moving), so each matmul streams 65 columns
  instead of 512 — ~2x fewer PE cycles for att@V. The softmax
  denominator arrives as column 64 (ones column of V_aug) and is now a
  per-partition scalar: reciprocal + tensor_scalar_mul, no partition
  broadcast. y is transposed back to y^T for the projection with PE
  transpose ops (128 cycles/tile) against a host-supplied identity.
* causal-diagonal masking moved to the (otherwise idle) GPSIMD engine.
* x^T, the mask and the identity are prepared on the host — no
  transpose DMAs on device.
"""

import math
from contextlib import ExitStack

import numpy as np
import ml_dtypes

import concourse.bass as bass
import concourse.mybir as mybir
from concourse import bacc
from concourse.bass_utils import run_bass_kernel_spmd
from concourse.tile import TileContext

B, T, C = 4, 2048, 1024
H, D = 16, 64
HL = 8            # heads per core
NCORES = 8
P = 128
CK = C // P       # 8 contraction chunks for the QKV projection
TB = 512          # t-block (query-block) width
NTB = T // TB     # 4
NTT = T // P      # 16 row tiles
QKC = 2 * HL * D  # 1024 q+k channels per core
VC = HL * D       # 512 v channels per core
PC = VC           # 512 proj contraction channels per core
WS = 32.0         # host-side scale on W_attn for the fp8 hi/lo split
PS = 32.0         # host-side scale on W_proj for the fp8 hi/lo split
YS = 16.0         # on-device scale on y^T for the fp8 hi/lo split

f32 = mybir.dt.float32
bf16 = mybir.dt.bfloat16
fp8 = mybir.dt.float8e4
BF = ml_dtypes.bfloat16
E4 = ml_dtypes.float8_e4m3


def _build_program():
    nc = bacc.Bacc("TRN2", target_bir_lowering=False, debug=False)
    xh = nc.dram_tensor("xh", (P, CK, T), fp8, kind="ExternalInput").ap()
    xl = nc.dram_tensor("xl", (P, CK, T), fp8, kind="ExternalInput").ap()
    wqh = nc.dram_tensor("wqh", (P, CK, 3 * VC), fp8, kind="ExternalInput").ap()
    wql = nc.dram_tensor("wql", (P, CK, 3 * VC), fp8, kind="ExternalInput").ap()
    bqk = nc.dram_tensor("bqk", (P, CK), f32, kind="ExternalInput").ap()
    wph = nc.dram_tensor("wph", (P, PC // P, C), fp8, kind="ExternalInput").ap()
    wpl = nc.dram_tensor("wpl", (P, PC // P, C), fp8, kind="ExternalInput").ap()
    mwm = nc.dram_tensor("mw", (P, P), bf16, kind="ExternalInput").ap()
    idn = nc.dram_tensor("idn", (P, P), bf16, kind="ExternalInput").ap()
    out = nc.dram_tensor("out", (T, C), bf16, kind="ExternalOutput").ap()

    # q,k in SBUF carry the 32x weight scale each -> scores are 1024x
    scale = 1.0 / math.sqrt(D) / (WS * WS)

    with TileContext(nc) as tc:
        with ExitStack() as ctx:
            const = ctx.enter_context(tc.tile_pool(name="const", bufs=1))
            persist = ctx.enter_context(tc.tile_pool(name="persist", bufs=1))

            mw = const.tile([P, P], bf16)
            ident = const.tile([P, P], bf16)
            bqk_sb = const.tile([P, CK], f32)

            # persistent SBUF tensors
            xh_sb = persist.tile([P, CK, T], fp8)          # x^T hi
            xl_sb = persist.tile([P, CK, T], fp8)          # x^T lo
            qkt = persist.tile([P, CK, T], bf16)           # (Q^T|K^T)*32 +bias
            vaug = persist.tile([P, NTT, HL, D + 1], bf16)  # V + ones col
            yth = persist.tile([P, HL // 2, T], fp8)       # 16*y^T hi
            ytl = persist.tile([P, HL // 2, T], fp8)       # 16*y^T lo
            wqh_sb = persist.tile([P, CK, 3 * VC], fp8)
            wql_sb = persist.tile([P, CK, 3 * VC], fp8)
            wph_sb = persist.tile([P, PC // P, C], fp8)
            wpl_sb = persist.tile([P, PC // P, C], fp8)

            nc.gpsimd.memset(vaug[:, :, :, D : D + 1], 1.0)
            warm = const.tile([P, 256], bf16)
            nc.gpsimd.memset(warm[:], 0.0)

            # x^T (hi+lo) for the first t-block goes first on the DMA pipes
            # (it gates the first matmul); W_attn streams behind in exact
            # consumption order (Q cols then K cols: the first two QKV tiles
            # are a Q tile and a K tile), hi and lo on separate queues so
            # each j-tile's weight pair lands together. Constants ride after
            # the tensors that gate earlier compute.
            nc.sync.dma_start(xh_sb[:, :, 0:TB], xh[:, :, 0:TB])
            nc.scalar.dma_start(xl_sb[:, :, 0:TB], xl[:, :, 0:TB])
            nc.gpsimd.dma_start(wqh_sb[:, :, VC : 2 * VC], wqh[:, :, VC : 2 * VC])
            nc.sync.dma_start(wqh_sb[:, :, 0:VC], wqh[:, :, 0:VC])
            nc.scalar.dma_start(wql_sb[:, :, 0:VC], wql[:, :, 0:VC])
            nc.gpsimd.dma_start(wql_sb[:, :, VC : 2 * VC], wql[:, :, VC : 2 * VC])
            nc.gpsimd.dma_start(bqk_sb[:], bqk)
            nc.sync.dma_start(wqh_sb[:, :, 2 * VC : 3 * VC], wqh[:, :, 2 * VC : 3 * VC])
            nc.scalar.dma_start(wql_sb[:, :, 2 * VC : 3 * VC], wql[:, :, 2 * VC : 3 * VC])
            nc.sync.dma_start(mw[:], mwm)
            nc.sync.dma_start(ident[:], idn)
            nc.sync.dma_start(wph_sb[:], wph)
            nc.scalar.dma_start(wpl_sb[:], wpl)

            with ExitStack() as c1:
                mm_psum = c1.enter_context(
                    tc.tile_pool(name="mm_psum", bufs=2, space="PSUM")
                )
                ps_psum = c1.enter_context(
                    tc.tile_pool(name="ps_psum", bufs=2, space="PSUM")
                )
                yq_psum = c1.enter_context(
                    tc.tile_pool(name="yq_psum", bufs=1, space="PSUM")
                )
                yt_psum = c1.enter_context(
                    tc.tile_pool(name="yt_psum", bufs=1, space="PSUM")
                )
                pt_pool = c1.enter_context(tc.tile_pool(name="pt", bufs=4))
                yn_pool = c1.enter_context(tc.tile_pool(name="yn", bufs=8))
                sm_pool = c1.enter_context(tc.tile_pool(name="sm", bufs=4))
                ot_pool = c1.enter_context(tc.tile_pool(name="ot", bufs=2))

                # warm-up matmuls on scratch data while the first DMAs are in
                # flight: the PE clock needs ~3.4us of sustained activity to
                # reach full rate.
                for _w in range(24):
                    wps = mm_psum.tile([P, 256], f32, tag="mm", name="wps")
                    nc.tensor.matmul(
                        wps[:], warm[:, 0:P], warm[:], start=True, stop=True
                    )

                def x_dma(tb):
                    def thunk():
                        nc.sync.dma_start(
                            xh_sb[:, :, tb * TB : (tb + 1) * TB],
                            xh[:, :, tb * TB : (tb + 1) * TB],
                        )
                        nc.sync.dma_start(
                            xl_sb[:, :, tb * TB : (tb + 1) * TB],
                            xl[:, :, tb * TB : (tb + 1) * TB],
                        )
                    return thunk

                def qk_tile(tb, j):
                    """One 128-channel Q/K tile of the fp8 3-term QKV matmul
                    for t-block tb (~1.3us of PE work)."""
                    def thunk():
                        tcols = slice(tb * TB, (tb + 1) * TB)
                        ps = mm_psum.tile([P, TB], f32, tag="mm")
                        first = True
                        for (a, b_) in ((0, 0), (0, 1), (1, 0)):
                            xs = (xh_sb, xl_sb)[a]
                            ws = (wqh_sb, wql_sb)[b_]
                            for g in range(CK // 2):
                                nc.tensor.matmul(
                                    ps[:],
                                    ws[:, 2 * g : 2 * g + 2, j * P : (j + 1) * P],
                                    xs[:, 2 * g : 2 * g + 2, tcols],
                                    start=first,
                                    stop=(a, b_, g) == (1, 0, CK // 2 - 1),
                                    perf_mode=mybir.MatmulPerfMode.DoubleRow,
                                )
                                first = False
                        nc.vector.tensor_scalar_add(
                            qkt[:, j, tcols], ps[:], bqk_sb[:, j : j + 1]
                        )
                    return thunk

                def v_tile(tb, ts4):
                    """One 128-row V tile (descaled into vaug)."""
                    def thunk():
                        tt = tb * (TB // P) + ts4
                        ps = mm_psum.tile([P, VC], f32, tag="mm")
                        first = True
                        for (a, b_) in ((0, 0), (0, 1), (1, 0)):
                            xs = (xh_sb, xl_sb)[a]
                            ws = (wqh_sb, wql_sb)[b_]
                            for g in range(CK // 2):
                                nc.tensor.matmul(
                                    ps[:],
                                    xs[:, 2 * g : 2 * g + 2, tt * P : (tt + 1) * P],
                                    ws[:, 2 * g : 2 * g + 2, QKC : QKC + VC],
                                    start=first,
                                    stop=(a, b_, g) == (1, 0, CK // 2 - 1),
                                    perf_mode=mybir.MatmulPerfMode.DoubleRow,
                                )
                                first = False
                        nc.vector.tensor_scalar_mul(
                            vaug[:, tt, :, 0:D],
                            ps[:].rearrange("p (h d) -> p h d", h=HL),
                            1.0 / WS,
                        )
                    return thunk

                def emit_head(j, h, fillers):
                    """Scores + exp + att@V for head h, query block j.

                    fillers: {pair_idx: [thunks]} — independent PE work
                    emitted at the START of that pair, so the PE has
                    something to chew on while the exp stream catches up
                    (the score-psum ring only lets the PE run 2 pairs
                    ahead of the ACT engine).

                    Returns a finisher (transpose + y^T hi/lo store) to be
                    injected as a filler into the next head.
                    """
                    nch = 4 * j + 4  # causal: key chunks 0..4j+3
                    npair = nch // 2
                    trail = min(2, npair - 1)

                    def dstart(c):
                        return max(0, (c - 4 * j) * P)

                    r0 = (h % 2) * D
                    qT = qkt[r0 : r0 + D, h // 2, :]
                    kT = qkt[r0 : r0 + D, 4 + h // 2, :]
                    yq = yq_psum.tile([P, 4, D + 1], f32, tag="yq")
                    pts = []

                    def attv(pb):
                        # The four qt accumulation chains share one PSUM bank.
                        # start=True marks the WHOLE bank pending-zero, so it
                        # must be issued exactly once (first matmul of the
                        # bank); the other chains' first writes then land in
                        # overwrite mode off the same bank-wide mark.
                        for ci, c in enumerate((2 * pb, 2 * pb + 1)):
                            for qt in range(4):
                                if c > 4 * j + qt:
                                    continue
                                q0 = qt * P
                                nc.tensor.matmul(
                                    yq[:, qt, :],
                                    pts[pb][:, ci, q0 : q0 + P],
                                    vaug[:, c, h, :],
                                    start=(c == 0 and qt == 0),
                                    stop=(c == 4 * j + qt),
                                    skip_group_check=True,
                                )

                    for pp in range(npair):
                        for f in fillers.get(pp, ()):
                            f()
                        c0, c1 = 2 * pp, 2 * pp + 1
                        dp = dstart(c0)
                        ps2 = ps_psum.tile([P, 2, TB], f32)
                        for ci, c in enumerate((c0, c1)):
                            dc = dstart(c)
                            nc.tensor.matmul(
                                ps2[:, ci, dc:],
                                kT[:, c * P : (c + 1) * P],
                                qT[:, j * TB + dc : (j + 1) * TB],
                                start=True,
                                stop=True,
                            )
                        pt = pt_pool.tile([P, 2, TB], bf16)
                        nc.scalar.activation(
                            pt[:, :, dp:], ps2[:, :, dp:],
                            mybir.ActivationFunctionType.Exp, scale=scale,
                        )
                        for ci, c in enumerate((c0, c1)):
                            d0 = dstart(c)
                            if (c - 4 * j) * P >= 0:
                                # zero key > query entries on the diagonal
                                nc.gpsimd.tensor_mul(
                                    pt[:, ci, d0 : d0 + P],
                                    pt[:, ci, d0 : d0 + P],
                                    mw[:],
                                )
                        pts.append(pt)
                        if pp >= trail:
                            attv(pp - trail)
                    for pb in range(npair - trail, npair):
                        attv(pb)

                    # normalize immediately: all yq readers are emitted before
                    # the next head re-requests the (bufs=1) yq buffer. The
                    # DVE chain runs during the next head's scores.
                    yns = []
                    for qt in range(4):
                        linv = sm_pool.tile([P, 1], f32, tag="linv")
                        nc.vector.reciprocal(linv[:], yq[:, qt, D : D + 1])
                        yn = yn_pool.tile([P, D], bf16, tag="yn")
                        nc.vector.tensor_scalar_mul(
                            yn[:], yq[:, qt, 0:D], linv[:]
                        )
                        yns.append(yn)

                    def finisher():
                        ytr = yt_psum.tile([D, 4, P], bf16, tag="yt")
                        for qt in range(4):
                            nc.tensor.matmul(
                                ytr[:, qt, :], yns[qt][:], ident[:],
                                is_transpose=True,
                            )
                        ycols = slice(j * TB, (j + 1) * TB)
                        src = ytr[:].rearrange("d q p -> d (q p)")
                        dsth = yth[r0 : r0 + D, h // 2, ycols]
                        nc.vector.tensor_scalar_mul(dsth, src, YS)
                        nc.vector.scalar_tensor_tensor(
                            ytl[r0 : r0 + D, h // 2, ycols],
                            src,
                            YS,
                            dsth,
                            op0=mybir.AluOpType.mult,
                            op1=mybir.AluOpType.subtract,
                        )

                    return finisher

                def make_proj_spacer(jb):
                    # emits one (t-tile, nh) slice of block jb's fp8 3-term
                    # projection per call; 8 calls cover the block
                    ots = {}

                    def spacer(g):
                        t4, nh = g // 2, g % 2
                        tt = 4 * jb + t4
                        if nh == 0:
                            ots[t4] = ot_pool.tile(
                                [P, C], bf16, name="ot", tag="ot"
                            )
                        ot = ots[t4]
                        po = mm_psum.tile([P, TB], f32, tag="mm", name="po")
                        first = True
                        for (a, b_) in ((0, 0), (0, 1), (1, 0)):
                            ys = (yth, ytl)[a]
                            ws = (wph_sb, wpl_sb)[b_]
                            for gg in range(PC // P // 2):
                                nc.tensor.matmul(
                                    po[:],
                                    ys[:, 2 * gg : 2 * gg + 2,
                                       tt * P : (tt + 1) * P],
                                    ws[:, 2 * gg : 2 * gg + 2,
                                       nh * TB : (nh + 1) * TB],
                                    start=first,
                                    stop=(a, b_, gg) == (1, 0, PC // P // 2 - 1),
                                    perf_mode=mybir.MatmulPerfMode.DoubleRow,
                                )
                                first = False
                        nc.vector.tensor_scalar_mul(
                            ot[:, nh * TB : (nh + 1) * TB], po[:],
                            1.0 / (YS * PS),
                        )
                        if jb == NTB - 1 and t4 == 3:
                            # last tile: store each half as soon as its copy
                            # lands so the final DMA is half-sized
                            nc.sync.dma_start(
                                out[tt * P : (tt + 1) * P,
                                    nh * TB : (nh + 1) * TB],
                                ot[:, nh * TB : (nh + 1) * TB],
                            )
                        elif nh == 1:
                            nc.sync.dma_start(
                                out[tt * P : (tt + 1) * P, :], ot[:]
                            )

                    def sp(g):
                        return lambda: spacer(g)

                    return sp

                pp_pool = c1.enter_context(tc.tile_pool(name="pp", bufs=16))
                partials = {}

                def make_proj_split(jb):
                    """Projection of block jb split by contraction half:
                    spa(g) contracts channel chunks 0,1 (heads 0..3) into a
                    bf16 partial — placeable as soon as those four heads'
                    finishers are in. spb(g) contracts chunks 2,3, adds the
                    partial and stores."""

                    def spa(g):
                        def thunk():
                            t4, nh = g // 2, g % 2
                            tt = 4 * jb + t4
                            po = mm_psum.tile([P, TB], f32, tag="mm", name="po")
                            first = True
                            for (a, b_) in ((0, 0), (0, 1), (1, 0)):
                                ys = (yth, ytl)[a]
                                ws = (wph_sb, wpl_sb)[b_]
                                nc.tensor.matmul(
                                    po[:],
                                    ys[:, 0:2, tt * P : (tt + 1) * P],
                                    ws[:, 0:2, nh * TB : (nh + 1) * TB],
                                    start=first,
                                    stop=(a, b_) == (1, 0),
                                    perf_mode=mybir.MatmulPerfMode.DoubleRow,
                                )
                                first = False
                            part = pp_pool.tile([P, TB], bf16, tag="part")
                            nc.vector.tensor_scalar_mul(
                                part[:], po[:], 1.0 / (YS * PS)
                            )
                            partials[(jb, g)] = part
                        return thunk

                    def spb(g):
                        def thunk():
                            t4, nh = g // 2, g % 2
                            tt = 4 * jb + t4
                            ot = ot_pool.tile([P, TB], bf16, name="ot", tag="ot3")
                            po = mm_psum.tile([P, TB], f32, tag="mm", name="po")
                            first = True
                            for (a, b_) in ((0, 0), (0, 1), (1, 0)):
                                ys = (yth, ytl)[a]
                                ws = (wph_sb, wpl_sb)[b_]
                                nc.tensor.matmul(
                                    po[:],
                                    ys[:, 2:4, tt * P : (tt + 1) * P],
                                    ws[:, 2:4, nh * TB : (nh + 1) * TB],
                                    start=first,
                                    stop=(a, b_) == (1, 0),
                                    perf_mode=mybir.MatmulPerfMode.DoubleRow,
                                )
                                first = False
                            nparts = 2 if (jb, g) == (NTB - 1, 7) else 1
                            w = TB // nparts
                            for half in range(nparts):
                                cs = slice(half * w, (half + 1) * w)
                                nc.vector.scalar_tensor_tensor(
                                    ot[:, cs],
                                    po[:, cs],
                                    1.0 / (YS * PS),
                                    partials[(jb, g)][:, cs],
                                    op0=mybir.AluOpType.mult,
                                    op1=mybir.AluOpType.add,
                                )
                                nc.sync.dma_start(
                                    out[tt * P : (tt + 1) * P,
                                        nh * TB + cs.start : nh * TB + cs.stop],
                                    ot[:, cs],
                                )
                        return thunk

                    return spa, spb

                sp0 = make_proj_spacer(0)
                sp1 = make_proj_spacer(1)
                sp2a, sp2b = make_proj_split(2)
                sp3a, sp3b = make_proj_split(3)

                # Static schedule. Heads of adjacent blocks are interleaved
                # so the exp stream never starves (block sequencing left the
                # ACT engine idle mid-kernel while late blocks starved the
                # PE of independent work). PE work that does not depend on
                # the exp stream (QKV tiles, V tiles, projection half-
                # contractions, the previous head's transposes) is chopped
                # into ~0.6-1.3us thunks and placed between score pairs.
                order = [
                    (0, 0), (0, 1), (0, 2), (0, 3),
                    (0, 4), (0, 5), (0, 6), (0, 7),
                    (1, 0), (1, 1), (1, 2), (1, 3),
                    (2, 0), (1, 4), (2, 1), (1, 5),
                    (2, 2), (1, 6), (2, 3), (1, 7),
                    (3, 0), (2, 4), (3, 1), (2, 5),
                    (3, 2), (2, 6), (3, 3), (2, 7),
                    (3, 4), (3, 5), (3, 6), (3, 7),
                ]
                plan = {
                    (0, 0): {1: [v_tile(0, 0), v_tile(0, 1),
                                 v_tile(0, 2), v_tile(0, 3)]},
                    (0, 1): {0: [qk_tile(0, 1)], 1: [qk_tile(0, 5)]},
                    (0, 2): {0: [qk_tile(0, 2)]},
                    (0, 3): {0: [qk_tile(0, 6)]},
                    (0, 4): {0: [x_dma(1), qk_tile(0, 3)]},
                    (0, 5): {0: [qk_tile(0, 7)]},
                    (0, 6): {0: [qk_tile(1, 0)], 1: [qk_tile(1, 4)]},
                    (0, 7): {0: [qk_tile(1, 1)], 1: [qk_tile(1, 5)]},
                    (1, 0): {0: [qk_tile(1, 2)], 1: [v_tile(1, 0)],
                             2: [v_tile(1, 1)],
                             3: [v_tile(1, 2), v_tile(1, 3)]},
                    (1, 1): {0: [qk_tile(1, 6)], 1: [qk_tile(1, 3)],
                             2: [qk_tile(1, 7)]},
                    (1, 2): {0: [x_dma(2)], 1: [qk_tile(2, 0)],
                             2: [qk_tile(2, 4)], 3: [qk_tile(2, 1)]},
                    (1, 3): {0: [qk_tile(2, 5)], 1: [qk_tile(2, 2)],
                             2: [qk_tile(2, 6)], 3: [qk_tile(2, 3)]},
                    (2, 0): {0: [qk_tile(2, 7)], 1: [v_tile(2, 0)],
                             2: [v_tile(2, 1)], 3: [v_tile(2, 2)],
                             4: [v_tile(2, 3)]},
                    (1, 4): {0: [x_dma(3)], 1: [qk_tile(3, 0)],
                             2: [qk_tile(3, 4)], 3: [qk_tile(3, 1)]},
                    (2, 1): {1: [qk_tile(3, 5)], 2: [qk_tile(3, 2)],
                             3: [qk_tile(3, 6)]},
                    (1, 5): {1: [qk_tile(3, 3)], 2: [qk_tile(3, 7)],
                             3: [v_tile(3, 0)]},
                    (2, 2): {1: [v_tile(3, 1)], 2: [v_tile(3, 2)],
                             3: [v_tile(3, 3)]},
                    (1, 6): {1: [sp0(0)], 2: [sp0(1)]},
                    (2, 3): {1: [sp0(2)], 3: [sp0(3)]},
                    (1, 7): {1: [sp0(4)], 2: [sp0(5)]},
                    (3, 0): {1: [sp0(6)], 3: [sp0(7)], 5: [sp1(0)]},
                    (2, 4): {1: [sp1(1)], 3: [sp1(2)]},
                    (3, 1): {1: [sp1(3)], 3: [sp1(4)], 5: [sp1(5)]},
                    (2, 5): {1: [sp1(6)], 3: [sp1(7)]},
                    (3, 2): {1: [sp2a(0)], 3: [sp2a(1)], 5: [sp2a(2)]},
                    (2, 6): {1: [sp2a(3)], 3: [sp2a(4)]},
                    (3, 3): {1: [sp2a(5)], 3: [sp2a(6)], 5: [sp2a(7)]},
                    (2, 7): {2: [sp3a(0)], 3: [sp3a(1)]},
                    (3, 4): {2: [sp2b(0)], 3: [sp2b(1)], 4: [sp2b(2)],
                             5: [sp2b(3)]},
                    (3, 5): {1: [sp2b(4)], 2: [sp2b(5)], 3: [sp2b(6)],
                             5: [sp3a(2)]},
                    (3, 6): {1: [sp3a(3)], 2: [sp3a(4)], 3: [sp3a(5)],
                             5: [sp3a(6)]},
                    (3, 7): {1: [sp2b(7)], 3: [sp3a(7)]},
                }

                qk_tile(0, 0)()
                qk_tile(0, 4)()
                fin = None
                for (j, h) in order:
                    fillers = {k: list(v) for k, v in plan[(j, h)].items()}
                    if fin is not None:
                        npair_h = 2 * (j + 1)
                        slot = min(1, npair_h - 1)
                        fillers.setdefault(slot, [])
                        fillers[slot].insert(0, fin)
                    fin = emit_head(j, h, fillers)
                fin()
                for g in range(8):
                    sp3b(g)()

    nc.compile()
    return nc


_NC_CACHE = None


def _get_program():
    global _NC_CACHE
    if _NC_CACHE is None:
        _NC_CACHE = _build_program()
    return _NC_CACHE


def _split_fp8(v):
    hi = v.astype(E4)
    lo = (v - hi.astype(np.float32)).astype(E4)
    return hi, lo


def _shard_inputs(x, W_attn, b_attn, bQ, bK, bV, W_proj):
    # weights/biases depend only on the head-half; build the two unique
    # variants once instead of once per core
    per_half = []
    for half in range(2):
        s = half * VC
        wq = W_attn[:, s : s + VC]
        wk = W_attn[:, C + s : C + s + VC]
        wv = W_attn[:, 2 * C + s : 2 * C + s + VC]
        wqkv = np.concatenate([wq, wk, wv], axis=1) * WS
        # [C, 3VC] -> [P, CK, 3VC] with channel a*128+p -> (p, a)
        wqkv = wqkv.reshape(CK, P, 3 * VC).transpose(1, 0, 2)
        wh, wl = _split_fp8(np.ascontiguousarray(wqkv))
        bq = b_attn[s : s + VC] + bQ[half * HL : half * HL + HL].reshape(-1)
        bk = b_attn[C + s : C + s + VC] + bK[half * HL : half * HL + HL].reshape(-1)
        bqk = np.ascontiguousarray(
            (np.concatenate([bq, bk]) * WS).reshape(CK, P).T.astype(np.float32)
        )
        wproj = np.ascontiguousarray(
            (W_proj[s : s + VC, :] * PS).reshape(PC // P, P, C)
            .transpose(1, 0, 2)
        )
        wph, wpl = _split_fp8(wproj)
        per_half.append(
            {"wqh": wh, "wql": wl, "bqk": bqk, "wph": wph, "wpl": wpl}
        )

    mask = np.triu(np.ones((P, P), np.float32)).astype(BF)  # mw[p,i]=1 iff i>=p
    ident = np.eye(P, dtype=np.float32).astype(BF)
    per_batch = []
    for b in range(B):
        # x[b] [T, C] -> x^T [P, CK, T] with channel a*128+p -> (p, a)
        xt = np.ascontiguousarray(
            x[b].T.reshape(CK, P, T).transpose(1, 0, 2)
        )
        xh, xlo = _split_fp8(xt)
        per_batch.append({"xh": xh, "xl": xlo})
    return [
        {**per_batch[c // 2], **per_half[c % 2], "mw": mask, "idn": ident}
        for c in range(NCORES)
    ]


def kernel(x, W_attn, b_attn, W_proj, b_proj, bQ, bK, bV, _trace=False, _res_out=None):
    x = np.asarray(x, dtype=np.float32)
    W_attn = np.asarray(W_attn, dtype=np.float32)
    b_attn = np.asarray(b_attn, dtype=np.float32)
    W_proj = np.asarray(W_proj, dtype=np.float32)
    b_proj = np.asarray(b_proj, dtype=np.float32)
    bQ = np.asarray(bQ, dtype=np.float32)
    bK = np.asarray(bK, dtype=np.float32)
    bV = np.asarray(bV, dtype=np.float32)

    nc = _get_program()
    in_maps = _shard_inputs(x, W_attn, b_attn, bQ, bK, bV, W_proj)
    res = run_bass_kernel_spmd(
        nc, in_maps, core_ids=list(range(NCORES)), trace=_trace
    )
    if _res_out is not None:
        _res_out.append(res)

    # v-bias passes through softmax untouched (rows of att sum to 1), so it
    # projects to a constant vector; fold it with b_proj on the host.
    bv = b_attn[2 * C : 3 * C] + bV.reshape(-1)
    extra = bv @ W_proj + b_proj
    out = np.empty((B, T, C), dtype=np.float32)
    for b in range(B):
        out[b] = (
            res.results[2 * b]["out"].astype(np.float32)
            + res.results[2 * b + 1]["out"].astype(np.float32)
            + extra
        )
    return out
